# revision 2
# baseline (speedup 1.0000x reference)
"""CrossAttention kernel for 8 Trainium2 NeuronCores — v2.

Sharding: core c handles batch b = c // 2 and head-group hg = c % 2
(8 of the 16 heads). Per-head attention needs no cross-device comms; the
out-projection partials are summed on the host (+ folded bias consts).

Math identities (vs the torch/jax reference):
  - softmax((q+bq)@(k+bk).T) == softmax((q+bq)@k.T): bk adds a per-row
    constant. bk never touches the device.
  - A @ (v + bv) == A @ v + bv (softmax rows sum to 1): bv folds into a
    host-side constant bv @ wo.T + bo added at the end.
  - scores |s| <~ 3 here, so exp() without max-subtraction is safe.

v2 design (vs v1): the whole schedule is built around keeping the ACT
(exp) stream and the PE stream simultaneously saturated.
  - Query superblock = 512 (one PSUM-bank-width). Scores for a head pair
    land in ONE [128, 2, 512] PSUM tile (2 banks), ping-ponged (bufs=2),
    so exp(tk) overlaps scores(tk+1) with no PSUM reuse stall.
  - One ACT exp call per (hp, tk): [128, 2x512] psum -> bf16 e-pair.
  - AV accumulators for the pair live in ONE [65, 2, 512] PSUM tile
    (2 banks, ones-column -> softmax denominators at partition 64).
  - PSUM budget: ss 2x2 + avs 2 + scratch 2x1 = 8 banks exactly; the
    scratch bank pair serves the deferred q/k/v/out-proj + norm items.
  - All projection / out-proj / norm work is deferred into a deadline-
    sorted queue drained one item per attention step, emitted BETWEEN
    scores(tk) and AV(tk-1) (AV waits on exp(tk-1), so queue items fill
    exactly the PE window where AV would stall).
  - Forced drains at each step guarantee producers are emitted before
    consumers (emission order = engine FIFO order; a late producer
    behind its consumer in the PE FIFO would deadlock).
  - DMAs are issued in column chunks prioritized so the first scores
    can start ~6us in, instead of after the full 8MB input load.
  - The two heads of a pair occupy PE row-groups 0-63/64-127
    (tile_position auto-derived), so their 64-contraction scores
    matmuls stream concurrently through the PE array.
"""

import sys

if "/opt/trn_rl_repo" not in sys.path:
    sys.path.insert(0, "/opt/trn_rl_repo")

from contextlib import ExitStack

import ml_dtypes
import numpy as np

B, LQ, LC, D, H = 4, 2048, 2048, 1024, 16
HD = D // H          # 64
DH = 512             # local head dims per core (8 heads)
P = 128
DT = D // P          # 8  k-tiles over the model dim
MT = DH // P         # 4  partition-tiles over local head dims (head pairs)
NH = 8               # local heads
TT = LC // P         # 16 key-pos tiles
TQ = 512             # query superblock = one psum bank width
NTX = LQ // TQ       # 4 superblocks

_CACHE: dict = {}


def _build_bass(n_hp=4, do_proj=True, do_outproj=True, do_av=True, do_exp=True,
                do_norm=True, loop_n=1):
    import concourse.bass as bass  # noqa: F401
    import concourse.mybir as mybir
    import concourse.tile as tile
    from concourse import bacc

    bf = mybir.dt.bfloat16
    f32 = mybir.dt.float32
    A = mybir.AluOpType
    EXP = mybir.ActivationFunctionType.Exp

    nc = bacc.Bacc(
        "TRN2",
        target_bir_lowering=False,
        debug=False,
        enable_asserts=False,
        num_devices=8,
    )

    xT = nc.dram_tensor("xT", [D, LQ], bf, kind="ExternalInput").ap()
    xcT = nc.dram_tensor("xcT", [D, LC], bf, kind="ExternalInput").ap()
    wqT = nc.dram_tensor("wqT", [D, DH], bf, kind="ExternalInput").ap()
    wkT = nc.dram_tensor("wkT", [D, DH], bf, kind="ExternalInput").ap()
    wvT = nc.dram_tensor("wvT", [D, DH], bf, kind="ExternalInput").ap()
    woT = nc.dram_tensor("woT", [DH, D], bf, kind="ExternalInput").ap()
    bq = nc.dram_tensor("bq", [P, MT], f32, kind="ExternalInput").ap()
    out = nc.dram_tensor("out", [LQ, D], f32, kind="ExternalOutput").ap()

    SPT = TT + 1                 # steps per (tx, hp) loop
    SPX = n_hp * SPT             # steps per superblock

    with tile.TileContext(nc) as tc, ExitStack() as ctx:
        const = ctx.enter_context(tc.tile_pool(name="const", bufs=1))
        xT_sb = const.tile([P, DT, LQ], bf, tag="xT")
        xcT_sb = const.tile([P, DT, LC], bf, tag="xcT")
        wq_sb = const.tile([P, DT, DH], bf, tag="wq")
        wk_sb = const.tile([P, DT, DH], bf, tag="wk")
        wv_sb = const.tile([P, DT, DH], bf, tag="wv")
        wo_sb = const.tile([P, MT, D], bf, tag="wo")
        bq_sb = const.tile([P, MT], f32, tag="bq")
        ones_sb = const.tile([1, HD], bf, tag="ones")
        ktp = const.tile([P, MT, LC], bf, tag="ktp")         # K^T
        vp = const.tile([P, TT, NH, HD + 1], bf, tag="vp")   # V + ones col

        nc.vector.memset(ones_sb[:], 1.0)
        nc.vector.memset(vp[:, :, :, HD : HD + 1], 1.0)

        # DMA plan: one strided DMA per tensor/column-chunk (folds the
        # 8 kt sub-loads into a single instruction: fewer sem updates,
        # >=2KB descriptor lines), split across the two HWDGE queues
        # (SP + Activation) and ordered by first use: kproj(0,0) needs
        # wk + xcT[:1024]; qproj(tx0,0) needs wq + xT[:512]; vproj(0..)
        # needs wv; the rest streams in behind the attention steps.
        xT3 = xT.rearrange("(t p) c -> p t c", p=P)
        xcT3 = xcT.rearrange("(t p) c -> p t c", p=P)
        wq3 = wqT.rearrange("(t p) c -> p t c", p=P)
        wk3 = wkT.rearrange("(t p) c -> p t c", p=P)
        wv3 = wvT.rearrange("(t p) c -> p t c", p=P)
        wo3 = woT.rearrange("(m p) c -> p m c", p=P)
        nc.sync.dma_start(out=wk_sb[:], in_=wk3)
        nc.scalar.dma_start(out=wq_sb[:], in_=wq3)
        nc.scalar.dma_start(out=bq_sb[:], in_=bq[:, :])
        nc.sync.dma_start(out=xcT_sb[:, :, 0:512], in_=xcT3[:, :, 0:512])
        nc.scalar.dma_start(out=xT_sb[:, :, 0:TQ], in_=xT3[:, :, 0:TQ])
        nc.scalar.dma_start(out=wv_sb[:], in_=wv3)
        nc.sync.dma_start(out=xcT_sb[:, :, 512:1024], in_=xcT3[:, :, 512:1024])
        nc.scalar.dma_start(out=xcT_sb[:, :, 1024:1536], in_=xcT3[:, :, 1024:1536])
        nc.sync.dma_start(out=xcT_sb[:, :, 1536:2048], in_=xcT3[:, :, 1536:2048])
        nc.scalar.dma_start(out=xT_sb[:, :, TQ:LQ], in_=xT3[:, :, TQ:LQ])
        nc.sync.dma_start(out=wo_sb[:], in_=wo3)

        psum = ctx.enter_context(tc.tile_pool(name="psum", bufs=1, space="PSUM"))
        epool = ctx.enter_context(tc.tile_pool(name="epool", bufs=4))
        qpool = ctx.enter_context(tc.tile_pool(name="qpool", bufs=2))
        apool = ctx.enter_context(tc.tile_pool(name="apool", bufs=2))
        spool = ctx.enter_context(tc.tile_pool(name="spool", bufs=4))
        upool = ctx.enter_context(tc.tile_pool(name="upool", bufs=4))
        opool = ctx.enter_context(tc.tile_pool(name="opool", bufs=3))

        def emit_body():
            # ---------- deferred work: deadline-sorted queue ----------
            sched = []  # list of (deadline_step, seq, fn), kept sorted
            seq_ctr = [0]

            def queue(deadline, fn):
                seq_ctr[0] += 1
                sched.append((deadline, seq_ctr[0], fn))
                sched.sort(key=lambda it: (it[0], it[1]))

            def drain(step, voluntary=1):
                # forced: everything due at or before this step
                while sched and sched[0][0] <= step:
                    sched.pop(0)[2]()
                    voluntary -= 1
                for _ in range(voluntary):
                    if not sched:
                        break
                    sched.pop(0)[2]()

            # ---------- work item constructors ----------
            def kproj_chunk(mt, nb):
                def run():
                    ps = psum.tile([P, TQ], f32, tag="scr", bufs=2,
                                   name=f"kp_{mt}_{nb}")
                    for kt in range(DT):
                        nc.tensor.matmul(
                            ps[:],
                            wk_sb[:, kt, mt * P : (mt + 1) * P],
                            xcT_sb[:, kt, nb * TQ : (nb + 1) * TQ],
                            start=(kt == 0),
                            stop=(kt == DT - 1),
                        )
                    nc.vector.tensor_copy(ktp[:, mt, nb * TQ : (nb + 1) * TQ], ps[:])
                return run

            def vproj_tile(tt):
                def run():
                    ps = psum.tile([P, DH], f32, tag="scr", bufs=2, name=f"vp_{tt}")
                    for kt in range(DT):
                        nc.tensor.matmul(
                            ps[:],
                            xcT_sb[:, kt, tt * P : (tt + 1) * P],
                            wv_sb[:, kt, :],
                            start=(kt == 0),
                            stop=(kt == DT - 1),
                        )
                    nc.vector.tensor_copy(
                        vp[:, tt, :, 0:HD],
                        ps[:].rearrange("p (h d) -> p h d", h=NH),
                    )
                return run

            def qproj_chunk(qt, tx, mt):
                def run():
                    ps = psum.tile([P, TQ], f32, tag="scr", bufs=2,
                                   name=f"qp_{tx}_{mt}")
                    for kt in range(DT):
                        nc.tensor.matmul(
                            ps[:],
                            wq_sb[:, kt, mt * P : (mt + 1) * P],
                            xT_sb[:, kt, tx * TQ : (tx + 1) * TQ],
                            start=(kt == 0),
                            stop=(kt == DT - 1),
                        )
                    nc.vector.tensor_scalar(
                        qt[:, mt, :], ps[:], bq_sb[:, mt : mt + 1], 0.125,
                        A.add, A.mult,
                    )
                return run

            def norm_item(at, avs, hp, h, tag):
                rc = spool.tile([1, TQ], bf, tag="rc", name=f"rc_{tag}")
                with nc.allow_low_precision(reason="1/denom in bf16: 0.4% scale noise, well within tolerance"):
                    nc.vector.reciprocal(rc[:], avs[HD : HD + 1, h, :])
                uh = upool.tile([HD, TQ], bf, tag="uh", name=f"uh_{tag}")
                nc.vector.tensor_copy(uh[:], avs[0:HD, h, :])

                def run():
                    pb = psum.tile([HD, TQ], f32, tag="scr", bufs=2,
                                   name=f"pb_{tag}")
                    nc.tensor.matmul(pb[:], ones_sb[:], rc[:], start=True, stop=True)
                    nc.vector.tensor_tensor(
                        at[h * HD : (h + 1) * HD, hp, :], uh[:], pb[:], op=A.mult
                    )
                return run

            def outproj_chunk(at, tx, ot, nb):
                def run():
                    ps = psum.tile([P, TQ], f32, tag="scr", bufs=2,
                                   name=f"op_{tx}_{ot}_{nb}")
                    for mt in range(MT):
                        nc.tensor.matmul(
                            ps[:],
                            at[:, mt, ot * P : (ot + 1) * P],
                            wo_sb[:, mt, nb * TQ : (nb + 1) * TQ],
                            start=(mt == 0),
                            stop=(mt == MT - 1),
                        )
                    ob = opool.tile([P, TQ], f32, tag="ob", name=f"ob_{tx}_{ot}_{nb}")
                    nc.vector.tensor_copy(ob[:], ps[:])
                    r0 = (tx * (TQ // P) + ot) * P
                    nc.sync.dma_start(
                        out=out[r0 : r0 + P, nb * TQ : (nb + 1) * TQ], in_=ob[:]
                    )
                return run

            # ---------- prefix: minimal inline work before attention ----
            qts = {}
            if do_proj:
                kproj_chunk(0, 0)()
                qt0 = qpool.tile([P, MT, TQ], bf, tag="qt", name="qt_0")
                qts[0] = qt0
                qproj_chunk(qt0, 0, 0)()
                vproj_tile(0)()
                vproj_tile(1)()
                # deadlines are "early but safe": consumption step minus
                # about half an hp-loop, so voluntary drains spread the
                # work and forced drains stay a rarely-hit safety net.
                for tt in range(2, TT):
                    queue(tt + 1, vproj_tile(tt))
                for nb in range(1, LC // TQ):
                    queue(4 * nb, kproj_chunk(0, nb))
                for mt in range(1, MT):
                    queue(max(0, mt * SPT - SPT // 2), kproj_chunk(mt, 0))
                    for nb in range(1, LC // TQ):
                        queue(mt * SPT + 4 * nb - SPT // 2, kproj_chunk(mt, nb))
                for mt in range(1, MT):
                    queue(max(0, mt * SPT - SPT // 2), qproj_chunk(qt0, 0, mt))
            else:
                nc.vector.memset(ktp[:], 0.0)
                nc.vector.memset(vp[:], 0.001)
                qt0 = qpool.tile([P, MT, TQ], bf, tag="qt", name="qt_0")
                qts[0] = qt0
                nc.vector.memset(qt0[:], 0.0)

            # ---------- attention superblocks ----------
            scored = set()
            es = {}

            def emit_scores_exp(qt_, tx_, hp_, tk_):
                if (tx_, hp_, tk_) in scored:
                    return
                scored.add((tx_, hp_, tk_))
                ss = psum.tile([P, 2, TQ], f32, tag="ss", bufs=2,
                               name=f"s_{tx_}_{hp_}_{tk_}")
                for h in range(2):
                    off = h * HD
                    nc.tensor.matmul(
                        ss[:, h, :],
                        ktp[off : off + HD, hp_, tk_ * P : (tk_ + 1) * P],
                        qt_[off : off + HD, hp_, :],
                        start=True,
                        stop=True,
                    )
                e = epool.tile([P, 2, TQ], bf, tag="e", name=f"e_{tx_}_{hp_}_{tk_}")
                if do_exp:
                    nc.scalar.activation(e[:], ss[:], EXP)
                else:
                    nc.vector.tensor_copy(e[:], ss[:])
                es[(tx_, hp_, tk_)] = e

            for tx in range(NTX):
                base = tx * SPX
                if tx in qts:
                    qt = qts.pop(tx)
                else:  # pragma: no cover (qt always prebuilt)
                    qt = qpool.tile([P, MT, TQ], bf, tag="qt", name=f"qt_{tx}")
                # queue next superblock's Q projection early
                if do_proj and tx + 1 < NTX:
                    nqt = qpool.tile([P, MT, TQ], bf, tag="qt", name=f"qt_{tx + 1}")
                    qts[tx + 1] = nqt
                    # drained during THIS superblock (qt is double-buffered,
                    # so the previous user's reads finished a superblock ago)
                    for mt in range(MT):
                        queue(base + mt * SPT + SPT // 2, qproj_chunk(nqt, tx + 1, mt))

                at = apool.tile([P, MT, TQ], bf, tag="at", name=f"at_{tx}")
                if n_hp < 4 or not (do_av and do_norm):
                    nc.vector.memset(at[:], 0.001)

                for hp in range(n_hp):
                    avs = None
                    for tk in range(SPT):
                        step = base + hp * SPT + tk
                        drain(step, voluntary=0)
                        if tk == 1 and do_av:
                            avs = psum.tile([HD + 1, 2, TQ], f32, tag="avs",
                                            bufs=1, name=f"av_{tx}_{hp}")
                        if tk < TT:
                            emit_scores_exp(qt, tx, hp, tk)
                        elif hp + 1 < n_hp:
                            emit_scores_exp(qt, tx, hp + 1, 0)
                        elif tx + 1 in qts:
                            emit_scores_exp(qts[tx + 1], tx + 1, 0, 0)
                        drain(step, voluntary=1)
                        if tk > 0 and do_av:
                            e = es.pop((tx, hp, tk - 1))
                            for h in range(2):
                                nc.tensor.matmul(
                                    avs[:, h, :],
                                    vp[:, tk - 1, 2 * hp + h, :],
                                    e[:, h, :],
                                    start=(tk - 1 == 0),
                                    stop=(tk - 1 == TT - 1),
                                )
                    if not (do_av and do_norm):
                        continue
                    for h in range(2):
                        # hp3's norm drifts into the next superblock's early
                        # steps instead of being forced at the boundary
                        queue(base + (hp + 1) * SPT + SPT // 2 + h,
                              norm_item(at, avs, hp, h, f"{tx}_{hp}_{h}"))

                # out-projection partials for this superblock, spread through
                # the next one (must finish before at(tx+2) is written)
                if do_outproj:
                    i = 0
                    for ot in range(TQ // P):
                        for nb in range(D // TQ):
                            queue(base + SPX + SPT + 4 * i, outproj_chunk(at, tx, ot, nb))
                            i += 1

            # tail: flush everything
            while sched:
                sched.pop(0)[2]()

        if loop_n > 1:
            with tc.For_i(0, loop_n, 1):
                emit_body()
        else:
            emit_body()

    nc.compile()
    return nc


def _get_nc(**kw):
    key = tuple(sorted(kw.items()))
    if key not in _CACHE:
        _CACHE[key] = _build_bass(**kw)
    return _CACHE[key]


def _prep_core_inputs(x_cond, x, wq, bq, wk, wv, wo):
    bfl = ml_dtypes.bfloat16
    maps = []
    for c in range(8):
        b, hg = divmod(c, 2)
        hs = slice(hg * DH, (hg + 1) * DH)
        maps.append(
            {
                "xT": np.ascontiguousarray(x[b].T).astype(bfl),
                "xcT": np.ascontiguousarray(x_cond[b].T).astype(bfl),
                "wqT": np.ascontiguousarray(wq[hs, :].T).astype(bfl),
                "wkT": np.ascontiguousarray(wk[hs, :].T).astype(bfl),
                "wvT": np.ascontiguousarray(wv[hs, :].T).astype(bfl),
                "woT": np.ascontiguousarray(wo[:, hs].T).astype(bfl),
                "bq": np.ascontiguousarray(
                    bq[hs].astype(np.float32).reshape(MT, P).T
                ),
            }
        )
    return maps


def kernel(x_cond, x, wq, bq, wk, bk, wv, bv, wo, bo):
    from concourse.bass_utils import run_bass_kernel_spmd

    x_cond = np.asarray(x_cond, np.float32)
    x = np.asarray(x, np.float32)
    wq, bq = np.asarray(wq, np.float32), np.asarray(bq, np.float32)
    wk = np.asarray(wk, np.float32)
    wv, bv = np.asarray(wv, np.float32), np.asarray(bv, np.float32)
    wo, bo = np.asarray(wo, np.float32), np.asarray(bo, np.float32)

    nc = _get_nc()
    in_maps = _prep_core_inputs(x_cond, x, wq, bq, wk, wv, wo)
    res = run_bass_kernel_spmd(nc, in_maps, list(range(8)))

    cvec = (
        bv.astype(np.float64) @ wo.T.astype(np.float64) + bo.astype(np.float64)
    ).astype(np.float32)
    full = np.empty((B, LQ, D), np.float32)
    for b in range(B):
        full[b] = res.results[2 * b]["out"] + res.results[2 * b + 1]["out"] + cvec
    return full


# revision 5
# speedup vs baseline: 4.7067x; 4.7067x over previous
"""CrossAttention kernel for 8 Trainium2 NeuronCores — v2.

Sharding: core c handles batch b = c // 2 and head-group hg = c % 2
(8 of the 16 heads). Per-head attention needs no cross-device comms; the
out-projection partials are summed on the host (+ folded bias consts).

Math identities (vs the torch/jax reference):
  - softmax((q+bq)@(k+bk).T) == softmax((q+bq)@k.T): bk adds a per-row
    constant. bk never touches the device.
  - A @ (v + bv) == A @ v + bv (softmax rows sum to 1): bv folds into a
    host-side constant bv @ wo.T + bo added at the end.
  - scores |s| <~ 3 here, so exp() without max-subtraction is safe.

v2 design (vs v1): the whole schedule is built around keeping the ACT
(exp) stream and the PE stream simultaneously saturated.
  - Query superblock = 512 (one PSUM-bank-width). Scores for a head pair
    land in ONE [128, 2, 512] PSUM tile (2 banks), ping-ponged (bufs=2),
    so exp(tk) overlaps scores(tk+1) with no PSUM reuse stall.
  - One ACT exp call per (hp, tk): [128, 2x512] psum -> bf16 e-pair.
  - AV accumulators for the pair live in ONE [65, 2, 512] PSUM tile
    (2 banks, ones-column -> softmax denominators at partition 64).
  - PSUM budget: ss 2x2 + avs 2 + scratch 2x1 = 8 banks exactly; the
    scratch bank pair serves the deferred q/k/v/out-proj + norm items.
  - All projection / out-proj / norm work is deferred into a deadline-
    sorted queue drained one item per attention step, emitted BETWEEN
    scores(tk) and AV(tk-1) (AV waits on exp(tk-1), so queue items fill
    exactly the PE window where AV would stall).
  - Forced drains at each step guarantee producers are emitted before
    consumers (emission order = engine FIFO order; a late producer
    behind its consumer in the PE FIFO would deadlock).
  - DMAs are issued in column chunks prioritized so the first scores
    can start ~6us in, instead of after the full 8MB input load.
  - The two heads of a pair occupy PE row-groups 0-63/64-127
    (tile_position auto-derived), so their 64-contraction scores
    matmuls stream concurrently through the PE array.
"""

import sys

if "/opt/trn_rl_repo" not in sys.path:
    sys.path.insert(0, "/opt/trn_rl_repo")

from contextlib import ExitStack

import ml_dtypes
import numpy as np

B, LQ, LC, D, H = 4, 2048, 2048, 1024, 16
HD = D // H          # 64
DH = 512             # local head dims per core (8 heads)
P = 128
DT = D // P          # 8  k-tiles over the model dim
MT = DH // P         # 4  partition-tiles over local head dims (head pairs)
NH = 8               # local heads
TT = LC // P         # 16 key-pos tiles
TQ = 512             # query superblock = one psum bank width
NTX = LQ // TQ       # 4 superblocks

_CACHE: dict = {}


def _build_bass(n_hp=4, do_proj=True, do_outproj=True, do_av=True, do_exp=True,
                do_norm=True, exp_bf=False, loop_n=1):
    import concourse.bass as bass  # noqa: F401
    import concourse.mybir as mybir
    import concourse.tile as tile
    from concourse import bacc

    bf = mybir.dt.bfloat16
    f32 = mybir.dt.float32
    A = mybir.AluOpType
    EXP = mybir.ActivationFunctionType.Exp

    nc = bacc.Bacc(
        "TRN2",
        target_bir_lowering=False,
        debug=False,
        enable_asserts=False,
        num_devices=8,
    )

    xT = nc.dram_tensor("xT", [D, LQ], bf, kind="ExternalInput").ap()
    xcT = nc.dram_tensor("xcT", [D, LC], bf, kind="ExternalInput").ap()
    wqT = nc.dram_tensor("wqT", [D, DH], bf, kind="ExternalInput").ap()
    wkT = nc.dram_tensor("wkT", [D, DH], bf, kind="ExternalInput").ap()
    wvT = nc.dram_tensor("wvT", [D, DH], bf, kind="ExternalInput").ap()
    woT = nc.dram_tensor("woT", [DH, D], bf, kind="ExternalInput").ap()
    bq = nc.dram_tensor("bq", [P, MT], f32, kind="ExternalInput").ap()
    # bf16 partials: host upcasts and sums; halves the out-DMA bytes
    out = nc.dram_tensor("out", [LQ, D], bf, kind="ExternalOutput").ap()

    SPT = TT + 1                 # steps per (tx, hp) loop
    SPX = n_hp * SPT             # steps per superblock

    with tile.TileContext(nc) as tc, ExitStack() as ctx:
        const = ctx.enter_context(tc.tile_pool(name="const", bufs=1))
        xT_sb = const.tile([P, DT, LQ], bf, tag="xT")
        xcT_sb = const.tile([P, DT, LC], bf, tag="xcT")
        wq_sb = const.tile([P, DT, DH], bf, tag="wq")
        wk_sb = const.tile([P, DT, DH], bf, tag="wk")
        wv_sb = const.tile([P, DT, DH], bf, tag="wv")
        wo_sb = const.tile([P, MT, D], bf, tag="wo")
        bq_sb = const.tile([P, MT], f32, tag="bq")
        ones_sb = const.tile([1, HD], bf, tag="ones")
        ktp = const.tile([P, MT, LC], bf, tag="ktp")         # K^T
        vp = const.tile([P, TT, NH, HD + 1], bf, tag="vp")   # V + ones col

        nc.vector.memset(ones_sb[:], 1.0)
        nc.vector.memset(vp[:, :, :, HD : HD + 1], 1.0)

        # DMA plan: one strided DMA per tensor/column-chunk (folds the
        # 8 kt sub-loads into a single instruction: fewer sem updates,
        # >=2KB descriptor lines), split across the two HWDGE queues
        # (SP + Activation) and ordered by first use: kproj(0,0) needs
        # wk + xcT[:1024]; qproj(tx0,0) needs wq + xT[:512]; vproj(0..)
        # needs wv; the rest streams in behind the attention steps.
        xT3 = xT.rearrange("(t p) c -> p t c", p=P)
        xcT3 = xcT.rearrange("(t p) c -> p t c", p=P)
        wq3 = wqT.rearrange("(t p) c -> p t c", p=P)
        wk3 = wkT.rearrange("(t p) c -> p t c", p=P)
        wv3 = wvT.rearrange("(t p) c -> p t c", p=P)
        wo3 = woT.rearrange("(m p) c -> p m c", p=P)
        nc.sync.dma_start(out=wk_sb[:], in_=wk3)
        nc.sync.dma_start(out=xcT_sb[:, :, 0:512], in_=xcT3[:, :, 0:512])
        nc.scalar.dma_start(out=wq_sb[:], in_=wq3)
        nc.scalar.dma_start(out=xT_sb[:, :, 0:TQ], in_=xT3[:, :, 0:TQ])
        nc.scalar.dma_start(out=bq_sb[:], in_=bq[:, :])
        nc.scalar.dma_start(out=wv_sb[:], in_=wv3)
        nc.sync.dma_start(out=xcT_sb[:, :, 512:1024], in_=xcT3[:, :, 512:1024])
        nc.scalar.dma_start(out=xcT_sb[:, :, 1024:1536], in_=xcT3[:, :, 1024:1536])
        nc.sync.dma_start(out=xcT_sb[:, :, 1536:2048], in_=xcT3[:, :, 1536:2048])
        nc.scalar.dma_start(out=xT_sb[:, :, TQ:LQ], in_=xT3[:, :, TQ:LQ])
        nc.sync.dma_start(out=wo_sb[:], in_=wo3)

        psum = ctx.enter_context(tc.tile_pool(name="psum", bufs=1, space="PSUM"))
        epool = ctx.enter_context(tc.tile_pool(name="epool", bufs=4))
        qpool = ctx.enter_context(tc.tile_pool(name="qpool", bufs=2))
        apool = ctx.enter_context(tc.tile_pool(name="apool", bufs=2))
        spool = ctx.enter_context(tc.tile_pool(name="spool", bufs=4))
        upool = ctx.enter_context(tc.tile_pool(name="upool", bufs=4))
        opool = ctx.enter_context(tc.tile_pool(name="opool", bufs=3))

        def emit_body():
            # ---------- deferred work: deadline-sorted queue ----------
            sched = []  # list of (deadline_step, seq, fn), kept sorted
            seq_ctr = [0]

            def queue(deadline, fn):
                seq_ctr[0] += 1
                sched.append((deadline, seq_ctr[0], fn))
                sched.sort(key=lambda it: (it[0], it[1]))

            def drain(step, voluntary=1):
                # forced: everything due at or before this step
                while sched and sched[0][0] <= step:
                    sched.pop(0)[2]()
                    voluntary -= 1
                for _ in range(voluntary):
                    if not sched:
                        break
                    sched.pop(0)[2]()

            # ---------- work item constructors ----------
            def kproj_chunk(mt, nb):
                def run():
                    ps = psum.tile([P, TQ], f32, tag="scr", bufs=2,
                                   name=f"kp_{mt}_{nb}")
                    for kt in range(DT):
                        nc.tensor.matmul(
                            ps[:],
                            wk_sb[:, kt, mt * P : (mt + 1) * P],
                            xcT_sb[:, kt, nb * TQ : (nb + 1) * TQ],
                            start=(kt == 0),
                            stop=(kt == DT - 1),
                        )
                    nc.vector.tensor_copy(ktp[:, mt, nb * TQ : (nb + 1) * TQ], ps[:])
                return run

            def vproj_tile(tt):
                def run():
                    ps = psum.tile([P, DH], f32, tag="scr", bufs=2, name=f"vp_{tt}")
                    for kt in range(DT):
                        nc.tensor.matmul(
                            ps[:],
                            xcT_sb[:, kt, tt * P : (tt + 1) * P],
                            wv_sb[:, kt, :],
                            start=(kt == 0),
                            stop=(kt == DT - 1),
                        )
                    nc.vector.tensor_copy(
                        vp[:, tt, :, 0:HD],
                        ps[:].rearrange("p (h d) -> p h d", h=NH),
                    )
                return run

            def qproj_chunk(qt, tx, mt):
                def run():
                    ps = psum.tile([P, TQ], f32, tag="scr", bufs=2,
                                   name=f"qp_{tx}_{mt}")
                    for kt in range(DT):
                        nc.tensor.matmul(
                            ps[:],
                            wq_sb[:, kt, mt * P : (mt + 1) * P],
                            xT_sb[:, kt, tx * TQ : (tx + 1) * TQ],
                            start=(kt == 0),
                            stop=(kt == DT - 1),
                        )
                    nc.vector.tensor_scalar(
                        qt[:, mt, :], ps[:], bq_sb[:, mt : mt + 1], 0.125,
                        A.add, A.mult,
                    )
                return run

            def norm_item(at, avs, hp, h, tag):
                rc = spool.tile([1, TQ], bf, tag="rc", name=f"rc_{tag}")
                with nc.allow_low_precision(reason="1/denom in bf16: 0.4% scale noise, well within tolerance"):
                    nc.vector.reciprocal(rc[:], avs[HD : HD + 1, h, :])
                uh = upool.tile([HD, TQ], bf, tag="uh", name=f"uh_{tag}")
                nc.vector.tensor_copy(uh[:], avs[0:HD, h, :])

                def run():
                    pb = psum.tile([HD, TQ], f32, tag="scr", bufs=2,
                                   name=f"pb_{tag}")
                    nc.tensor.matmul(pb[:], ones_sb[:], rc[:], start=True, stop=True)
                    nc.vector.tensor_tensor(
                        at[h * HD : (h + 1) * HD, hp, :], uh[:], pb[:], op=A.mult
                    )
                return run

            def outproj_chunk(at, tx, ot, nb):
                def run():
                    ps = psum.tile([P, TQ], f32, tag="scr", bufs=2,
                                   name=f"op_{tx}_{ot}_{nb}")
                    for mt in range(MT):
                        nc.tensor.matmul(
                            ps[:],
                            at[:, mt, ot * P : (ot + 1) * P],
                            wo_sb[:, mt, nb * TQ : (nb + 1) * TQ],
                            start=(mt == 0),
                            stop=(mt == MT - 1),
                        )
                    ob = opool.tile([P, TQ], bf, tag="ob", name=f"ob_{tx}_{ot}_{nb}")
                    nc.vector.tensor_copy(ob[:], ps[:])
                    r0 = (tx * (TQ // P) + ot) * P
                    nc.sync.dma_start(
                        out=out[r0 : r0 + P, nb * TQ : (nb + 1) * TQ], in_=ob[:]
                    )
                return run

            # ---------- prefix: minimal inline work before attention ----
            qts = {}
            if do_proj:
                kproj_chunk(0, 0)()
                qt0 = qpool.tile([P, MT, TQ], bf, tag="qt", name="qt_0")
                qts[0] = qt0
                qproj_chunk(qt0, 0, 0)()
                # deadlines are "early but safe": consumption step minus
                # about half an hp-loop, so voluntary drains spread the
                # work and forced drains stay a rarely-hit safety net.
                for tt in range(TT):
                    queue(max(1, tt + 1), vproj_tile(tt))
                for nb in range(1, LC // TQ):
                    queue(4 * nb, kproj_chunk(0, nb))
                for mt in range(1, MT):
                    queue(max(0, mt * SPT - SPT // 2), kproj_chunk(mt, 0))
                    for nb in range(1, LC // TQ):
                        queue(mt * SPT + 4 * nb - SPT // 2, kproj_chunk(mt, nb))
                for mt in range(1, MT):
                    queue(max(0, mt * SPT - SPT // 2), qproj_chunk(qt0, 0, mt))
            else:
                nc.vector.memset(ktp[:], 0.0)
                nc.vector.memset(vp[:], 0.001)
                for _tx in range(NTX):
                    qts[_tx] = qpool.tile(
                        [P, MT, TQ], bf, tag="qt", name=f"qt_{_tx}"
                    )
                    nc.vector.memset(qts[_tx][:], 0.0)

            # ---------- attention superblocks ----------
            scored = set()
            es = {}

            def emit_scores_exp(qt_, tx_, hp_, tk_):
                if (tx_, hp_, tk_) in scored:
                    return
                scored.add((tx_, hp_, tk_))
                ss = psum.tile([P, 2, TQ], f32, tag="ss", bufs=2,
                               name=f"s_{tx_}_{hp_}_{tk_}")
                for h in range(2):
                    off = h * HD
                    nc.tensor.matmul(
                        ss[:, h, :],
                        ktp[off : off + HD, hp_, tk_ * P : (tk_ + 1) * P],
                        qt_[off : off + HD, hp_, :],
                        start=True,
                        stop=True,
                    )
                e = epool.tile([P, 2, TQ], bf, tag="e", name=f"e_{tx_}_{hp_}_{tk_}")
                if do_exp and exp_bf:
                    # stage scores to bf16 SBUF first: probes whether the
                    # ACT engine streams 16-bit input at 2x
                    eb = epool.tile([P, 2, TQ], bf, tag="eb",
                                    name=f"eb_{tx_}_{hp_}_{tk_}")
                    nc.vector.tensor_copy(eb[:], ss[:])
                    nc.scalar.activation(e[:], eb[:], EXP)
                elif do_exp:
                    nc.scalar.activation(e[:], ss[:], EXP)
                else:
                    nc.vector.tensor_copy(e[:], ss[:])
                es[(tx_, hp_, tk_)] = e

            for tx in range(NTX):
                base = tx * SPX
                if tx in qts:
                    qt = qts.pop(tx)
                else:  # pragma: no cover (qt always prebuilt)
                    qt = qpool.tile([P, MT, TQ], bf, tag="qt", name=f"qt_{tx}")
                # queue next superblock's Q projection early
                if do_proj and tx + 1 < NTX:
                    nqt = qpool.tile([P, MT, TQ], bf, tag="qt", name=f"qt_{tx + 1}")
                    qts[tx + 1] = nqt
                    # drained during THIS superblock (qt is double-buffered,
                    # so the previous user's reads finished a superblock ago)
                    for mt in range(MT):
                        queue(base + mt * SPT + SPT // 2, qproj_chunk(nqt, tx + 1, mt))

                at = apool.tile([P, MT, TQ], bf, tag="at", name=f"at_{tx}")
                if n_hp < 4 or not (do_av and do_norm):
                    nc.vector.memset(at[:], 0.001)

                for hp in range(n_hp):
                    avs = None
                    for tk in range(SPT):
                        step = base + hp * SPT + tk
                        drain(step, voluntary=0)
                        if tk == 1 and do_av:
                            avs = psum.tile([HD + 1, 2, TQ], f32, tag="avs",
                                            bufs=1, name=f"av_{tx}_{hp}")
                        if tk < TT:
                            emit_scores_exp(qt, tx, hp, tk)
                        elif hp + 1 < n_hp:
                            emit_scores_exp(qt, tx, hp + 1, 0)
                        elif tx + 1 in qts:
                            emit_scores_exp(qts[tx + 1], tx + 1, 0, 0)
                        drain(step, voluntary=1)
                        if tk > 0 and do_av:
                            e = es.pop((tx, hp, tk - 1))
                            for h in range(2):
                                nc.tensor.matmul(
                                    avs[:, h, :],
                                    vp[:, tk - 1, 2 * hp + h, :],
                                    e[:, h, :],
                                    start=(tk - 1 == 0),
                                    stop=(tk - 1 == TT - 1),
                                )
                    if not (do_av and do_norm):
                        continue
                    for h in range(2):
                        # hp3's norm drifts into the next superblock's early
                        # steps instead of being forced at the boundary
                        queue(base + (hp + 1) * SPT + SPT // 2 + h,
                              norm_item(at, avs, hp, h, f"{tx}_{hp}_{h}"))

                # out-projection partials for this superblock, spread through
                # the next one (must finish before at(tx+2) is written)
                if do_outproj:
                    i = 0
                    for ot in range(TQ // P):
                        for nb in range(D // TQ):
                            queue(base + SPX + SPT + 4 * i, outproj_chunk(at, tx, ot, nb))
                            i += 1

            # tail: flush everything
            while sched:
                sched.pop(0)[2]()

        if loop_n > 1:
            with tc.For_i(0, loop_n, 1):
                emit_body()
        else:
            emit_body()

    nc.compile()
    return nc


def _get_nc(**kw):
    key = tuple(sorted(kw.items()))
    if key not in _CACHE:
        _CACHE[key] = _build_bass(**kw)
    return _CACHE[key]


def _prep_core_inputs(x_cond, x, wq, bq, wk, wv, wo):
    bfl = ml_dtypes.bfloat16
    maps = []
    for c in range(8):
        b, hg = divmod(c, 2)
        hs = slice(hg * DH, (hg + 1) * DH)
        maps.append(
            {
                "xT": np.ascontiguousarray(x[b].T).astype(bfl),
                "xcT": np.ascontiguousarray(x_cond[b].T).astype(bfl),
                "wqT": np.ascontiguousarray(wq[hs, :].T).astype(bfl),
                "wkT": np.ascontiguousarray(wk[hs, :].T).astype(bfl),
                "wvT": np.ascontiguousarray(wv[hs, :].T).astype(bfl),
                "woT": np.ascontiguousarray(wo[:, hs].T).astype(bfl),
                "bq": np.ascontiguousarray(
                    bq[hs].astype(np.float32).reshape(MT, P).T
                ),
            }
        )
    return maps


def kernel(x_cond, x, wq, bq, wk, bk, wv, bv, wo, bo):
    from concourse.bass_utils import run_bass_kernel_spmd

    x_cond = np.asarray(x_cond, np.float32)
    x = np.asarray(x, np.float32)
    wq, bq = np.asarray(wq, np.float32), np.asarray(bq, np.float32)
    wk = np.asarray(wk, np.float32)
    wv, bv = np.asarray(wv, np.float32), np.asarray(bv, np.float32)
    wo, bo = np.asarray(wo, np.float32), np.asarray(bo, np.float32)

    nc = _get_nc()
    in_maps = _prep_core_inputs(x_cond, x, wq, bq, wk, wv, wo)
    res = run_bass_kernel_spmd(nc, in_maps, list(range(8)))

    cvec = (
        bv.astype(np.float64) @ wo.T.astype(np.float64) + bo.astype(np.float64)
    ).astype(np.float32)
    full = np.empty((B, LQ, D), np.float32)
    for b in range(B):
        full[b] = (
            res.results[2 * b]["out"].astype(np.float32)
            + res.results[2 * b + 1]["out"].astype(np.float32)
            + cvec
        )
    return full
